# revision 28
# baseline (speedup 1.0000x reference)
"""Distributed ISTFT kernel for Trainium2 (8 NeuronCores, Bass/Tile).

Math (matches the jax reference):
  z: [2, 513, T] one-sided spectrum (real/imag), T = 8192 frames.
  Hermitian extension + ifft(1024) + window + overlap-add (hop 256) +
  divide by overlapped window sum + trim 512 each side -> [2, 2096896].

Key folds used here:
  * real(ifft) = A^T @ X where A [1024(k), 1024(n)] packs the cos rows for
    zr bins 0..512 and sin rows for zi bins 1..511; X packs those z rows.
  * imag(ifft)[n, t] = (zi[0,t] + (-1)^n zi[512,t]) / N  (rank-2).
  * Output sample m = 256*b + r; block b = sum_{q=0..3} wf_{b-q}[256q+r].
    Folding window * A and the reciprocal window-sum into the stationary
    operand gives O^T[t, r] = sum_q X[:, t+3-q]^T @ Aw_q directly -- the
    overlap-add, windowing and normalization all ride inside the matmul.
  * Frame axis is sharded 1024 output blocks/core with a 3-frame input
    halo, so no cross-core communication is needed at all.  The two
    blocks whose window-sum misses a frame (global block 2 and 8192) get
    a data-driven row fixup (masks make the same program a no-op on the
    other cores).
"""

import numpy as np

N_FFT = 1024
HOP = 256
T_FRAMES = 8192
N_CORES = 8
F_SLOTS = 1027  # frame slots per core: 1024 owned blocks need slots t..t+3
NB = 1024       # output blocks computed per core (core 7 uses 1023)

_CACHE = {}


def _amat() -> np.ndarray:
    """A [1024(kappa), 1024(n)]: ifft cos/sin weights, f32."""
    n = np.arange(N_FFT, dtype=np.float64)[None, :]
    k = np.arange(513, dtype=np.float64)[:, None]
    g = np.full((513, 1), 2.0)
    g[0, 0] = 1.0
    g[512, 0] = 1.0
    C = (g / N_FFT) * np.cos(2.0 * np.pi * k * n / N_FFT)
    k2 = np.arange(1, 512, dtype=np.float64)[:, None]
    S = (-2.0 / N_FFT) * np.sin(2.0 * np.pi * k2 * n / N_FFT)
    return np.ascontiguousarray(np.concatenate([C, S], 0).astype(np.float32))


def _consts() -> np.ndarray:
    c = np.zeros((8, 256), np.float32)
    c[0:4, :] = (1.0 - 2.0 * (np.arange(256) % 2)).astype(np.float32)[None, :]
    c[4:8, :] = 1.0
    return c


def _build_nc():
    from contextlib import ExitStack

    import concourse.tile as tile
    from concourse import bacc, mybir

    f32 = mybir.dt.float32
    f32r = mybir.dt.float32r

    nc = bacc.Bacc("TRN2", target_bir_lowering=False, debug=False,
                   num_devices=N_CORES)

    # x is pre-encoded to fp32r on the host (fp32 with the mantissa
    # rounded to 11 bits -- the PE's fp32r operand precision), so its DMA
    # is a valid fp32r producer and no on-device conversion is needed.
    x_d = nc.dram_tensor("x", [1026, F_SLOTS], f32r, kind="ExternalInput")
    a_d = nc.dram_tensor("amat", [1024, 1024], f32, kind="ExternalInput")
    w_d = nc.dram_tensor("wvec", [1, N_FFT], f32, kind="ExternalInput")
    c_d = nc.dram_tensor("consts", [8, 256], f32, kind="ExternalInput")
    m_d = nc.dram_tensor("masks", [1, 2], f32, kind="ExternalInput")
    # aux row 0 = ones, row 1 = fp32r-rounded window (both fp32r-encoded)
    x2_d = nc.dram_tensor("aux", [2, N_FFT], f32r, kind="ExternalInput")
    o_d = nc.dram_tensor("out", [2, NB, 256], f32, kind="ExternalOutput")

    with tile.TileContext(nc) as tc, ExitStack() as ctx:
        big = ctx.enter_context(tc.tile_pool(name="big", bufs=1))
        stg = ctx.enter_context(tc.tile_pool(name="stg", bufs=4))
        sml = ctx.enter_context(tc.tile_pool(name="sml", bufs=1))
        # bank budget: 6 (ps0) + 2 (transient wbf psum, then ps1) = 8
        ps0p = ctx.enter_context(tc.tile_pool(name="ps0p", bufs=6, space="PSUM"))
        osb = ctx.enter_context(tc.tile_pool(name="osb", bufs=8))
        drm = ctx.enter_context(tc.tile_pool(name="drm", bufs=1, space="DRAM"))

        # ---- small setup inputs first (cheap), then the big streams.
        # aux rows lead: the window-broadcast matmuls gate everything.
        onesr = sml.tile([1, N_FFT], f32r, tag="onesr")
        nc.sync.dma_start(out=onesr[:], in_=x2_d.ap()[0:1, :])
        wror = sml.tile([1, N_FFT], f32r, tag="wror")
        nc.sync.dma_start(out=wror[:], in_=x2_d.ap()[1:2, :])
        w4 = sml.tile([4, 256], f32, tag="w4")
        nc.sync.dma_start(out=w4[:],
                          in_=w_d.ap().rearrange("a (b c) -> (a b) c", c=256))
        asn = sml.tile([4, 256], f32, tag="asn")
        nc.sync.dma_start(out=asn[:], in_=c_d.ap()[0:4, :])
        wrowf = sml.tile([1, N_FFT], f32, tag="wrowf")
        nc.sync.dma_start(out=wrowf[:], in_=w_d.ap())
        msk = sml.tile([1, 2], f32, tag="msk")
        nc.gpsimd.dma_start(out=msk[:], in_=m_d.ap())

        # raw-window broadcast [128, 1024] built by two K=1 fp32r matmuls
        # (ones x wrow) -- far faster than a DMA partition-broadcast.
        # The k=0 Aw quarters read it straight from psum so they skip the
        # SBUF copy; the copy then serves k=1..7.
        wbf = sml.tile([128, N_FFT], f32, tag="wbf")
        psw_cm = tc.tile_pool(name="psw", bufs=1, space="PSUM")
        psw = psw_cm.__enter__()
        ps_wb = psw.tile([128, N_FFT], f32, tag="ps_wb")
        nc.tensor.matmul(ps_wb[:, 0:512], lhsT=onesr[0:1, 0:128],
                         rhs=wror[0:1, 0:512], start=True, stop=True)
        nc.tensor.matmul(ps_wb[:, 512:1024], lhsT=onesr[0:1, 0:128],
                         rhs=wror[0:1, 512:1024], start=True, stop=True)

        # shifted zi[0], zi[512] rows on the scalar queue (issue cost of 8
        # small DMAs would otherwise sit mid-stream on the sync queue)
        tu = sml.tile([4, NB], f32r, tag="tu")
        tv = sml.tile([4, NB], f32r, tag="tv")
        for q in range(4):
            nc.gpsimd.dma_start(out=tu[q:q + 1, :],
                                in_=x_d.ap()[1024:1025, 3 - q:3 - q + NB])
            nc.gpsimd.dma_start(out=tv[q:q + 1, :],
                                in_=x_d.ap()[1025:1026, 3 - q:3 - q + NB])

        # early, dependency-free memsets
        rfx0 = sml.tile([128, 256], f32, tag="rfx0")
        nc.vector.memset(rfx0[:], 1.0)
        rfx7 = sml.tile([128, 256], f32, tag="rfx7")
        nc.vector.memset(rfx7[:], 1.0)

        # ---- big loads on the sync queue + Aw multiplies on DVE ----
        from concourse.tile import add_dep_helper

        xs = []
        aw = []
        aw_insts = []

        def load_k(k):
            xk = big.tile([128, F_SLOTS], f32r, tag=f"xs{k}", name=f"xs{k}")
            nc.sync.dma_start(out=xk[:], in_=x_d.ap()[128 * k:128 * (k + 1), :])
            xs.append(xk)
            ak_f = stg.tile([128, N_FFT], f32, tag="astg", name=f"astg{k}")
            awk = big.tile([128, N_FFT], f32r, tag=f"aw{k}", name=f"aw{k}")
            if k == 0:
                # quarter the first A chunk so the first Aw quarter (and
                # with it the whole matmul pipeline) is ready ~5us sooner
                mi = None
                for q in range(4):
                    cols = slice(256 * q, 256 * (q + 1))
                    nc.scalar.dma_start(out=ak_f[:, cols],
                                        in_=a_d.ap()[0:128, cols])
                    mi = nc.vector.tensor_mul(awk[:, cols], ak_f[:, cols],
                                              ps_wb[:, cols])
            else:
                nc.scalar.dma_start(out=ak_f[:],
                                    in_=a_d.ap()[128 * k:128 * (k + 1), :])
                mi = nc.vector.tensor_mul(awk[:], ak_f[:], wbf[:])
            aw.append(awk)
            aw_insts.append(mi)

        def after(inst, k, why):
            # order-only edge: keep this DVE op behind the k-th Aw multiply
            # so the scheduler cannot hoist it ahead of the matmul path
            add_dep_helper(inst.ins, aw_insts[k].ins, sync=False, reason=why)

        load_k(0)
        nc.vector.tensor_copy(wbf[:], ps_wb[:])
        psw_cm.__exit__(None, None, None)
        ps1p = ctx.enter_context(tc.tile_pool(name="ps1p", bufs=2, space="PSUM"))

        # window-sum chain: ws4/ws3a/ws3b packed in one row, one reciprocal
        wall = sml.tile([1, 768], f32, tag="wall")
        t01 = sml.tile([1, 256], f32, tag="t01")
        i0 = nc.vector.tensor_add(t01[:], wrowf[:, 0:256], wrowf[:, 256:512])
        after(i0, 0, "wsum a after aw0")
        t23 = sml.tile([1, 256], f32, tag="t23")
        nc.vector.tensor_add(t23[:], wrowf[:, 512:768], wrowf[:, 768:1024])
        nc.vector.tensor_add(wall[:, 0:256], t01[:], t23[:])
        i1 = nc.vector.tensor_sub(wall[:, 256:512], wall[:, 0:256],
                                  wrowf[:, 768:1024])
        after(i1, 0, "ws3a after aw0")
        i2 = nc.vector.tensor_sub(wall[:, 512:768], wall[:, 0:256],
                                  wrowf[:, 0:256])
        after(i2, 0, "ws3b after aw0")

        load_k(1)
        rall = sml.tile([1, 768], f32, tag="rall")
        i3 = nc.vector.reciprocal(rall[:], wall[:])
        after(i3, 1, "rcp after aw1")

        rwsd = drm.tile([1, 256], f32, tag="rwsd")
        nc.gpsimd.dma_start(out=rwsd[:], in_=rall[0:1, 0:256])
        rwsb = sml.tile([128, 256], f32, tag="rwsb")
        nc.gpsimd.dma_start(
            out=rwsb[:], in_=rwsd[0:1, :].partition_broadcast(128)[:, 0, :])

        load_k(2)
        load_k(3)

        # fixup factors f = 1 + mask * (ws4/ws3x - 1) -> full-height norms
        f0 = sml.tile([1, 256], f32, tag="f0")
        i4 = nc.vector.tensor_mul(f0[:], wall[0:1, 0:256], rall[0:1, 256:512])
        after(i4, 3, "f0 after aw3")
        nc.vector.tensor_scalar_sub(f0[:], f0[:], 1.0)
        nc.vector.tensor_scalar_mul(f0[:], f0[:], msk[0:1, 0:1])
        nc.vector.tensor_scalar_add(f0[:], f0[:], 1.0)
        f7 = sml.tile([1, 256], f32, tag="f7")
        i5 = nc.vector.tensor_mul(f7[:], wall[0:1, 0:256], rall[0:1, 512:768])
        after(i5, 3, "f7 after aw3")
        nc.vector.tensor_scalar_sub(f7[:], f7[:], 1.0)
        nc.vector.tensor_scalar_mul(f7[:], f7[:], msk[0:1, 1:2])
        nc.vector.tensor_scalar_add(f7[:], f7[:], 1.0)

        nc.vector.tensor_copy(rfx0[0:1, :], f0[:])
        nc.gpsimd.dma_start(out=rfx7[126:127, :], in_=f7[:])

        load_k(4)

        nrm0 = sml.tile([128, 256], f32, tag="nrm0")
        i6 = nc.vector.tensor_mul(nrm0[:], rwsb[:], rfx0[:])
        after(i6, 4, "nrm0 after aw4")
        nrm7 = sml.tile([128, 256], f32, tag="nrm7")
        i7 = nc.vector.tensor_mul(nrm7[:], rwsb[:], rfx7[:])
        after(i7, 4, "nrm7 after aw4")

        # channel-1 taps: raw window / N (eviction applies 1/ws4 once)
        tpu = sml.tile([4, 256], f32r, tag="tpu")
        nc.scalar.mul(tpu[:], w4[:], 1.0 / N_FFT)
        tpv_f = sml.tile([4, 256], f32, tag="tpv_f")
        i8 = nc.vector.tensor_mul(tpv_f[:], w4[:], asn[:])
        after(i8, 2, "tpv_f after aw2")
        tpv = sml.tile([4, 256], f32r, tag="tpv")
        nc.scalar.mul(tpv[:], tpv_f[:], 1.0 / N_FFT)

        for k in range(5, 8):
            load_k(k)

        def norm_for(tt):
            return nrm0 if tt == 0 else (nrm7 if tt == 7 else rwsb)

        def evict(ps, tt, ch):
            o = osb.tile([128, 256], f32, tag=f"o{ch}", name=f"o{ch}_{tt}")
            nc.vector.tensor_mul(o[:], ps[:], norm_for(tt)[:])
            nc.sync.dma_start(
                out=o_d.ap()[ch:ch + 1, tt * 128:(tt + 1) * 128, :], in_=o[:])

        def ch1_group(tt):
            ps1 = ps1p.tile([128, 256], f32, tag="ps1", name=f"ps1_{tt}")
            nc.tensor.matmul(ps1[:], lhsT=tu[:, tt * 128:tt * 128 + 128],
                             rhs=tpu[:], start=True, stop=False)
            nc.tensor.matmul(ps1[:], lhsT=tv[:, tt * 128:tt * 128 + 128],
                             rhs=tpv[:], start=False, stop=True)
            evict(ps1, tt, 1)

        # ---- channel 0: k-outer accumulation in two psum sweeps; the
        # tiny channel-1 groups fill the PE's DMA-pacing gaps in sweep 1
        def sweep(tts, ch1_sched):
            pss = {
                tt: ps0p.tile([128, 256], f32, tag="ps0", name=f"ps0_{tt}")
                for tt in tts
            }
            for k in range(8):
                for tt in tts:
                    for q in range(4):
                        off = tt * 128 + 3 - q
                        nc.tensor.matmul(
                            pss[tt][:],
                            lhsT=xs[k][:, off:off + 128],
                            rhs=aw[k][:, 256 * q:256 * (q + 1)],
                            start=(k == 0 and q == 0),
                            stop=(k == 7 and q == 3))
                for c1 in ch1_sched.get(k, []):
                    ch1_group(c1)
            for tt in tts:
                evict(pss[tt], tt, 0)

        sweep([0, 1, 2, 3, 4, 5], {})
        sweep([6, 7], {k: [k] for k in range(8)})

    nc.compile()
    return nc


def _inputs_for_cores(z: np.ndarray, window: np.ndarray):
    amat = _CACHE.get("amat")
    if amat is None:
        amat = _amat()
        _CACHE["amat"] = amat
    consts = _CACHE.get("consts")
    if consts is None:
        consts = _consts()
        _CACHE["consts"] = consts
    w4 = np.ascontiguousarray(window.reshape(1, 1024).astype(np.float32))
    aux = np.empty((2, 1024), np.float32)
    aux[0] = 1.0
    aux[1] = window.astype(np.float32)
    u = aux.view(np.uint32)
    u += np.uint32(0x800)
    u &= np.uint32(0xFFFFF000)
    aux = np.ascontiguousarray(aux)

    in_maps = []
    for c in range(N_CORES):
        G = 1024 * c - 1  # global frame index of slot 0
        X = np.zeros((1026, F_SLOTS), np.float32)
        lo, hi = max(0, G), min(T_FRAMES, G + F_SLOTS)
        s0, s1 = lo - G, hi - G
        X[0:513, s0:s1] = z[0, :, lo:hi]
        X[513:1024, s0:s1] = z[1, 1:512, lo:hi]
        X[1024, s0:s1] = z[1, 0, lo:hi]
        X[1025, s0:s1] = z[1, 512, lo:hi]
        # pre-encode to fp32r: round the fp32 mantissa to 11 bits, which
        # is what the PE's fp32r operand path keeps
        u = X.view(np.uint32)
        u += np.uint32(0x800)
        u &= np.uint32(0xFFFFF000)
        masks = np.array([[1.0 if c == 0 else 0.0,
                           1.0 if c == N_CORES - 1 else 0.0]], np.float32)
        in_maps.append({
            "x": X,
            "amat": amat,
            "wvec": w4,
            "consts": consts,
            "masks": masks,
            "aux": aux,
        })
    return in_maps


def kernel(z: np.ndarray, window: np.ndarray) -> np.ndarray:
    from concourse.bass_utils import run_bass_kernel_spmd

    z = np.asarray(z, dtype=np.float32)
    window = np.asarray(window, dtype=np.float32)

    nc = _CACHE.get("nc")
    if nc is None:
        nc = _build_nc()
        _CACHE["nc"] = nc

    in_maps = _inputs_for_cores(z, window)
    res = run_bass_kernel_spmd(nc, in_maps, list(range(N_CORES)))

    parts = []
    for c in range(N_CORES):
        nb = NB if c < N_CORES - 1 else NB - 1
        o = res.results[c]["out"]  # [2, NB, 256]
        parts.append(o[:, :nb, :].reshape(2, -1))
    return np.ascontiguousarray(np.concatenate(parts, axis=1))
